# revision 18
# baseline (speedup 1.0000x reference)
"""Trainium2 Bass kernel for nn_MultiHeadCDGCN.

Math (per batch b):
  t_w  = softmax(x, axis=T);  TAtt = sum_T(x * t_w)          [N, D]
  Q    = x @ W_Q.T                                           [T, N, D]
  K    = TAtt @ W_K.T ; V = TAtt @ W_V.T                     [N, D]
  S_th = Q_th @ K_h.T / sqrt(dh)   (per t, head h)           [N, N]
  out  = (relu(S) + I) @ V = relu(S) @ V + V                 [T, N, D]

Sharding: data-parallel over B across 8 NeuronCores (B == 8, one batch
per core); no collectives.

Key layout choices (v3):
  - Host supplies x.T ([D, T*N], bf16) per core and receives out.T
    ([D, T*N], fp32) plus V; the final transpose and the +V self-loop
    term run on the CPU. On-device this removes all transposes of x,
    the output stream-transpose, and the whole +V evacuation pass: the
    A@V PSUM tiles are DMA'd straight to HBM.
  - Q.T is evacuated from PSUM to SBUF by DMA (no ACT/DVE copy).
  - fp32 matmuls run as float32r; the Q projection and A@V run bf16.
  - S lands in bf16 PSUM tiles of 4 heads ([128, 2048], one bank pair)
    so relu evacuation runs half the instructions and the DVE tiles hit
    the 2-byte 2x path.
  - Softmax-pool stats are bf16 (exp on ACT, x*e mul on DVE 2x, e-sum
    on Pool, xe-sum on DVE) accumulating into fp32.
"""

import sys

import numpy as np

sys.path.insert(0, "/opt/trn_rl_repo")

import concourse.bacc as bacc  # noqa: E402
import concourse.tile as tile  # noqa: E402
from concourse import mybir  # noqa: E402
from concourse.bass_utils import run_bass_kernel_spmd  # noqa: E402

F32 = mybir.dt.float32
F32R = mybir.dt.float32r
BF16 = mybir.dt.bfloat16
AF = mybir.ActivationFunctionType
ALU = mybir.AluOpType

B, T, N, D, H, DH = 8, 32, 256, 256, 8, 32
P = 128
NCHUNKS = 16  # tn chunks of 512 (2 frames each)
CHUNK_T = 2  # frames per chunk
CHUNK_TN = CHUNK_T * N  # 512

_CACHE: dict = {}

# relu evacuation round-robin: 0=ACT, 1=DVE (Pool cannot access PSUM).
_RELU_SCHED = [0, 1, 0, 1, 0, 1, 0, 1]


def _build_program():
    nc = bacc.Bacc()

    xt_d = nc.dram_tensor("xt", [D, T * N], BF16, kind="ExternalInput")
    wqt_d = nc.dram_tensor("wqt", [D, D], BF16, kind="ExternalInput")
    wkt_d = nc.dram_tensor("wkt", [D, D], F32, kind="ExternalInput")
    wvt_d = nc.dram_tensor("wvt", [D, D], F32, kind="ExternalInput")
    out_d = nc.dram_tensor("out", [D, T * N], F32, kind="ExternalOutput")
    v_d = nc.dram_tensor("v", [2, P, D], F32, kind="ExternalOutput")

    xt_v = xt_d.rearrange("(dc p) tn -> p dc tn", p=P)
    out_v = out_d.rearrange("(hg p) tn -> p hg tn", p=P)

    with tile.TileContext(nc) as tc:
        with (
            tc.tile_pool(name="consts", bufs=1) as consts,
            tc.tile_pool(name="ew", bufs=3) as e_pool,
            tc.tile_pool(name="at", bufs=12) as a_pool,
            tc.tile_pool(name="ot", bufs=4) as o_pool,
            tc.tile_pool(name="misc", bufs=1) as misc,
            tc.tile_pool(name="ps_a", bufs=3, space="PSUM") as ps_a,
            tc.tile_pool(name="ps_o", bufs=2, space="PSUM") as ps_o,
        ):
            # Weights, [k, j] with k split over 2 partition tiles.
            wqt_sb = consts.tile([P, 2, D], BF16)
            wkt_sb = consts.tile([P, 2, D], F32R)
            wvt_sb = consts.tile([P, 2, D], F32R)
            for w_sb, w_d in ((wqt_sb, wqt_d), (wkt_sb, wkt_d), (wvt_sb, wvt_d)):
                nc.sync.dma_start(
                    out=w_sb,
                    in_=w_d.rearrange("(kc p) j -> p kc j", p=P).bitcast(
                        w_sb.dtype
                    ),
                )

            # x.T resident: [d%128, d//128, tn] (bf16).
            xt_all = consts.tile([P, 2, T * N], BF16)

            # Q.T strip [j, tn] resident (j split over 2 partition tiles),
            # cast to bf16 during PSUM evacuation so S runs as bf16.
            qt_sb = consts.tile([P, 2, T * N], BF16)

            # Softmax-pool statistics, frames of a chunk kept separate:
            # [d%128, d//128, (ti n)], fp32 accumulators. Ping-pong pairs:
            # an in-place (out==in) DVE/Pool tensor_add runs ~2.3x slower
            # than one with distinct output, so alternate two buffers.
            se_pp = [
                consts.tile([P, 2, CHUNK_TN], F32, name=f"se{i}")
                for i in range(2)
            ]
            sxe_pp = [
                consts.tile([P, 2, CHUNK_TN], F32, name=f"sxe{i}")
                for i in range(2)
            ]
            nc.gpsimd.memset(se_pp[0], 0.0)
            nc.gpsimd.memset(sxe_pp[0], 0.0)

            # ---------------- Phase A: stream x.T, stats, Q.T
            for c in range(NCHUNKS):
                cs = slice(c * CHUNK_TN, (c + 1) * CHUNK_TN)
                xt = xt_all[:, :, cs]
                nc.sync.dma_start(out=xt, in_=xt_v[:, :, cs])

                e_t = e_pool.tile([P, 2, CHUNK_TN], BF16, name="e_t")
                nc.scalar.activation(e_t, xt, AF.Exp)
                xe_t = e_pool.tile([P, 2, CHUNK_TN], BF16, name="xe_t")
                nc.vector.tensor_mul(xe_t, xt, e_t)
                # e-sum on Pool (SBUF-only), xe-sum on DVE.
                src_i, dst_i = c % 2, 1 - c % 2
                nc.gpsimd.tensor_add(se_pp[dst_i], se_pp[src_i], e_t)
                nc.vector.tensor_add(sxe_pp[dst_i], sxe_pp[src_i], xe_t)

                # Q.T chunk: [j, tn] = sum_k W_Q.T[k, j]^T x.T[k, tn]
                for jc in range(2):
                    pq = ps_a.tile([P, CHUNK_TN], F32, tag="psa", name=f"pq{jc}")
                    for kc in range(2):
                        nc.tensor.matmul(
                            pq,
                            wqt_sb[:, kc, jc * P : (jc + 1) * P],
                            xt[:, kc, :],
                            start=(kc == 0),
                            stop=(kc == 1),
                        )
                    # Both Q.T evacuations on ACT: DVE is the phase-A
                    # bottleneck (2-input stats ops), ACT has slack.
                    nc.scalar.activation(qt_sb[:, jc, cs], pq, AF.Copy)

            # ---------------- Phase B: TAtt.T, K.T, V
            # Fold the two frame-lanes of the stats, then TAtt = sxe/se.
            sum2_e = se_pp[1 - (NCHUNKS - 1) % 2]
            sum2_xe = sxe_pp[1 - (NCHUNKS - 1) % 2]
            sum_e = misc.tile([P, 2, N], F32)
            sum_xe = misc.tile([P, 2, N], F32)
            nc.vector.tensor_add(sum_e, sum2_e[:, :, :N], sum2_e[:, :, N:])
            nc.scalar.activation(sum_xe, sum2_xe[:, :, :N], AF.Copy)
            nc.vector.tensor_add(sum_xe, sum_xe, sum2_xe[:, :, N:])
            rec = misc.tile([P, 2, N], F32)
            tatt_t = consts.tile([P, 2, N], F32R)  # TAtt.T [d, n]
            nc.vector.reciprocal(rec, sum_e)
            nc.vector.tensor_mul(tatt_t, sum_xe, rec)

            kt_sb = consts.tile([P, 2, N], BF16)  # K.T [j, m] (pre-scaled)
            for jc in range(2):
                pk = ps_a.tile([P, N], F32, tag="psa", name="pk")
                for kc in range(2):
                    nc.tensor.matmul(
                        pk,
                        wkt_sb[:, kc, jc * P : (jc + 1) * P],
                        tatt_t[:, kc, :],
                        start=(kc == 0),
                        stop=(kc == 1),
                    )
                nc.vector.tensor_copy(kt_sb[:, jc, :], pk)

            v_sb = consts.tile([P, 2, D], BF16)  # V [m, j] (A@V stationary)
            for mc in range(2):
                pv = ps_a.tile([P, D], F32, tag="psa", name="pv")
                for kc in range(2):
                    nc.tensor.matmul(
                        pv,
                        tatt_t[:, kc, mc * P : (mc + 1) * P],
                        wvt_sb[:, kc, :],
                        start=(kc == 0),
                        stop=(kc == 1),
                    )
                nc.vector.tensor_copy(v_sb[:, mc, :], pv)
                # Ship V to the host for the +V self-loop term (DMA cannot
                # read PSUM, so stage through SBUF).
                v_stage = misc.tile([P, D], F32, name=f"vs{mc}")
                nc.scalar.activation(v_stage, pv, AF.Copy)
                nc.sync.dma_start(out=v_d[mc], in_=v_stage)

            # ---------------- Phase C: attention + output
            relu_rr = 0
            for c in range(NCHUNKS):
                cs = slice(c * CHUNK_TN, (c + 1) * CHUNK_TN)
                a_str = {}
                for hg in range(2):
                    for mc in range(2):
                        for rp in range(2):  # head pairs share a 2-bank tile
                            ps2 = ps_a.tile(
                                [P, 2 * CHUNK_TN],
                                F32,
                                tag="psa",
                                name=f"ps{hg}{mc}{rp}",
                            )
                            for rh in range(2):
                                r = rp * 2 + rh
                                nc.tensor.matmul(
                                    ps2[:, rh * CHUNK_TN : (rh + 1) * CHUNK_TN],
                                    kt_sb[
                                        r * 32 : (r + 1) * 32,
                                        hg,
                                        mc * P : (mc + 1) * P,
                                    ],
                                    qt_sb[r * 32 : (r + 1) * 32, hg, cs],
                                    start=True,
                                    stop=True,
                                    tile_position=(r * 32, 0),
                                )
                            a2 = a_pool.tile(
                                [P, 2 * CHUNK_TN],
                                BF16,
                                tag="at",
                                name=f"a{hg}{mc}{rp}",
                            )
                            sel = _RELU_SCHED[relu_rr % len(_RELU_SCHED)]
                            relu_rr += 1
                            if sel == 0:
                                nc.scalar.activation(a2, ps2, AF.Relu)
                            else:
                                nc.vector.tensor_scalar_max(a2, ps2, 0.0)
                            for rh in range(2):
                                a_str[(hg, rp * 2 + rh, mc)] = a2[
                                    :, rh * CHUNK_TN : (rh + 1) * CHUNK_TN
                                ]
                for hg in range(2):
                    po = ps_o.tile([P, CHUNK_TN], F32, tag="po", name=f"po{hg}")
                    # All four column tiles accumulate concurrently into
                    # disjoint partition quadrants of one PSUM bank.
                    for mc in range(2):
                        for r in range(4):
                            h = hg * 4 + r
                            nc.tensor.matmul(
                                po[r * 32 : (r + 1) * 32, :],
                                v_sb[:, mc, h * 32 : (h + 1) * 32],
                                a_str[(hg, r, mc)],
                                start=(mc == 0),
                                stop=(mc == 1),
                                tile_position=(0, r * 32),
                                skip_group_check=True,
                            )
                    # Plain PSUM->SBUF copy (the host adds the +V term),
                    # then contiguous DMA to HBM.
                    o_sb = o_pool.tile([P, CHUNK_TN], F32, tag="ot", name=f"o{hg}")
                    if hg == 0:
                        nc.scalar.activation(o_sb, po, AF.Copy)
                    else:
                        nc.vector.tensor_copy(o_sb, po)
                    dma_eng = nc.gpsimd if hg == 0 else nc.sync
                    dma_eng.dma_start(out=out_v[:, hg, cs], in_=o_sb)

    nc.finalize()
    return nc


def prepare_in_maps(inputs):
    x = np.asarray(inputs["x"], dtype=np.float32)
    w_q = np.asarray(inputs["W_Q"], dtype=np.float32)
    w_k = np.asarray(inputs["W_K"], dtype=np.float32)
    w_v = np.asarray(inputs["W_V"], dtype=np.float32)

    import ml_dtypes

    wqt = np.ascontiguousarray(w_q.T).astype(ml_dtypes.bfloat16)
    wkt = np.ascontiguousarray(w_k.T) * np.float32(1.0 / np.sqrt(DH))
    wvt = np.ascontiguousarray(w_v.T)

    return [
        {
            "xt": np.ascontiguousarray(x[b].reshape(T * N, D).T).astype(
                ml_dtypes.bfloat16
            ),
            "wqt": wqt,
            "wkt": wkt,
            "wvt": wvt,
        }
        for b in range(B)
    ]


def finish_out(res):
    # out.T [D, T*N] -> [T, N, D] per core (+ V self-loop), stacked over B.
    outs = []
    for b in range(B):
        o = res.results[b]["out"].reshape(D, T, N).transpose(1, 2, 0)
        v = res.results[b]["v"].reshape(N, D)
        outs.append(o + v[None, :, :])
    return np.stack(outs, axis=0)


def kernel(**inputs) -> np.ndarray:
    if "nc" not in _CACHE:
        _CACHE["nc"] = _build_program()
    nc = _CACHE["nc"]
    in_maps = prepare_in_maps(inputs)
    res = run_bass_kernel_spmd(nc, in_maps, core_ids=list(range(B)))
    return finish_out(res)


# revision 19
# speedup vs baseline: 1.0384x; 1.0384x over previous
"""Trainium2 Bass kernel for nn_MultiHeadCDGCN.

Math (per batch b):
  t_w  = softmax(x, axis=T);  TAtt = sum_T(x * t_w)          [N, D]
  Q    = x @ W_Q.T                                           [T, N, D]
  K    = TAtt @ W_K.T ; V = TAtt @ W_V.T                     [N, D]
  S_th = Q_th @ K_h.T / sqrt(dh)   (per t, head h)           [N, N]
  out  = (relu(S) + I) @ V = relu(S) @ V + V                 [T, N, D]

Sharding: data-parallel over B across 8 NeuronCores (B == 8, one batch
per core); no collectives.

Key layout choices (v3):
  - Host supplies x.T ([D, T*N], bf16) per core and receives out.T
    ([D, T*N], fp32) plus V; the final transpose and the +V self-loop
    term run on the CPU. On-device this removes all transposes of x,
    the output stream-transpose, and the whole +V evacuation pass: the
    A@V PSUM tiles are DMA'd straight to HBM.
  - Q.T is evacuated from PSUM to SBUF by DMA (no ACT/DVE copy).
  - fp32 matmuls run as float32r; the Q projection and A@V run bf16.
  - S lands in bf16 PSUM tiles of 4 heads ([128, 2048], one bank pair)
    so relu evacuation runs half the instructions and the DVE tiles hit
    the 2-byte 2x path.
  - Softmax-pool stats are bf16 (exp on ACT, x*e mul on DVE 2x, e-sum
    on Pool, xe-sum on DVE) accumulating into fp32.
"""

import sys

import numpy as np

sys.path.insert(0, "/opt/trn_rl_repo")

import concourse.bacc as bacc  # noqa: E402
import concourse.tile as tile  # noqa: E402
from concourse import mybir  # noqa: E402
from concourse.bass_utils import run_bass_kernel_spmd  # noqa: E402

F32 = mybir.dt.float32
F32R = mybir.dt.float32r
BF16 = mybir.dt.bfloat16
AF = mybir.ActivationFunctionType
ALU = mybir.AluOpType

B, T, N, D, H, DH = 8, 32, 256, 256, 8, 32
P = 128
NCHUNKS = 16  # tn chunks of 512 (2 frames each)
CHUNK_T = 2  # frames per chunk
CHUNK_TN = CHUNK_T * N  # 512

_CACHE: dict = {}

# relu evacuation round-robin: 0=ACT, 1=DVE (Pool cannot access PSUM).
_RELU_SCHED = [0, 1, 0, 1, 0, 1, 0, 1]


def _build_program():
    nc = bacc.Bacc()

    xt_d = nc.dram_tensor("xt", [D, T * N], BF16, kind="ExternalInput")
    wqt_d = nc.dram_tensor("wqt", [D, D], BF16, kind="ExternalInput")
    wkt_d = nc.dram_tensor("wkt", [D, D], F32, kind="ExternalInput")
    wvt_d = nc.dram_tensor("wvt", [D, D], F32, kind="ExternalInput")
    out_d = nc.dram_tensor("out", [D, T * N], F32, kind="ExternalOutput")
    v_d = nc.dram_tensor("v", [2, P, D], F32, kind="ExternalOutput")

    xt_v = xt_d.rearrange("(dc p) tn -> p dc tn", p=P)
    out_v = out_d.rearrange("(hg p) tn -> p hg tn", p=P)

    with tile.TileContext(nc) as tc:
        with (
            tc.tile_pool(name="consts", bufs=1) as consts,
            tc.tile_pool(name="ew", bufs=3) as e_pool,
            tc.tile_pool(name="at", bufs=12) as a_pool,
            tc.tile_pool(name="ot", bufs=4) as o_pool,
            tc.tile_pool(name="misc", bufs=1) as misc,
            tc.tile_pool(name="ps_a", bufs=3, space="PSUM") as ps_a,
            tc.tile_pool(name="ps_o", bufs=2, space="PSUM") as ps_o,
        ):
            # Weights, [k, j] with k split over 2 partition tiles.
            wqt_sb = consts.tile([P, 2, D], BF16)
            wkt_sb = consts.tile([P, 2, D], F32R)
            wvt_sb = consts.tile([P, 2, D], F32R)
            for w_sb, w_d in ((wqt_sb, wqt_d), (wkt_sb, wkt_d), (wvt_sb, wvt_d)):
                nc.sync.dma_start(
                    out=w_sb,
                    in_=w_d.rearrange("(kc p) j -> p kc j", p=P).bitcast(
                        w_sb.dtype
                    ),
                )

            # x.T resident: [d%128, d//128, tn] (bf16).
            xt_all = consts.tile([P, 2, T * N], BF16)

            # Q.T strip [j, tn] resident (j split over 2 partition tiles),
            # cast to bf16 during PSUM evacuation so S runs as bf16.
            qt_sb = consts.tile([P, 2, T * N], BF16)

            # Softmax-pool statistics, frames of a chunk kept separate:
            # [d%128, d//128, (ti n)]. All-bf16 accumulation: a DVE
            # tensor_add with any fp32 operand runs ~2.8us per [f1024]
            # vs ~0.7-1.2us all-bf16. Two interleaved accumulators per
            # stat (8 adds each) bound the bf16 rounding accumulation;
            # the fold to fp32 happens once in phase B.
            se_pp = [
                consts.tile([P, 2, CHUNK_TN], BF16, name=f"se{i}")
                for i in range(2)
            ]
            sxe_pp = [
                consts.tile([P, 2, CHUNK_TN], BF16, name=f"sxe{i}")
                for i in range(2)
            ]
            for i in range(2):
                nc.gpsimd.memset(se_pp[i], 0.0)
                nc.gpsimd.memset(sxe_pp[i], 0.0)

            # ---------------- Phase A: stream x.T, stats, Q.T
            for c in range(NCHUNKS):
                cs = slice(c * CHUNK_TN, (c + 1) * CHUNK_TN)
                xt = xt_all[:, :, cs]
                nc.sync.dma_start(out=xt, in_=xt_v[:, :, cs])

                e_t = e_pool.tile([P, 2, CHUNK_TN], BF16, name="e_t")
                nc.scalar.activation(e_t, xt, AF.Exp)
                xe_t = e_pool.tile([P, 2, CHUNK_TN], BF16, name="xe_t")
                nc.vector.tensor_mul(xe_t, xt, e_t)
                # e-sum on Pool (SBUF-only), xe-sum on DVE.
                nc.gpsimd.tensor_add(se_pp[c % 2], se_pp[c % 2], e_t)
                nc.vector.tensor_add(sxe_pp[c % 2], sxe_pp[c % 2], xe_t)

                # Q.T chunk: [j, tn] = sum_k W_Q.T[k, j]^T x.T[k, tn]
                for jc in range(2):
                    pq = ps_a.tile([P, CHUNK_TN], F32, tag="psa", name=f"pq{jc}")
                    for kc in range(2):
                        nc.tensor.matmul(
                            pq,
                            wqt_sb[:, kc, jc * P : (jc + 1) * P],
                            xt[:, kc, :],
                            start=(kc == 0),
                            stop=(kc == 1),
                        )
                    # Both Q.T evacuations on ACT: DVE is the phase-A
                    # bottleneck (2-input stats ops), ACT has slack.
                    nc.scalar.activation(qt_sb[:, jc, cs], pq, AF.Copy)

            # ---------------- Phase B: TAtt.T, K.T, V
            # Fold the two frame-lanes of the stats, then TAtt = sxe/se.
            # Fold the two bf16 accumulators and the two frame lanes into
            # fp32 sums: (a0+a1) in bf16-in/fp32-out adds.
            sum2_e = misc.tile([P, 2, CHUNK_TN], F32)
            sum2_xe = misc.tile([P, 2, CHUNK_TN], F32)
            nc.vector.tensor_add(sum2_e, se_pp[0], se_pp[1])
            nc.gpsimd.tensor_add(sum2_xe, sxe_pp[0], sxe_pp[1])
            sum_e = misc.tile([P, 2, N], F32)
            sum_xe = misc.tile([P, 2, N], F32)
            nc.vector.tensor_add(sum_e, sum2_e[:, :, :N], sum2_e[:, :, N:])
            nc.scalar.activation(sum_xe, sum2_xe[:, :, :N], AF.Copy)
            nc.vector.tensor_add(sum_xe, sum_xe, sum2_xe[:, :, N:])
            rec = misc.tile([P, 2, N], F32)
            tatt_t = consts.tile([P, 2, N], F32R)  # TAtt.T [d, n]
            nc.vector.reciprocal(rec, sum_e)
            nc.vector.tensor_mul(tatt_t, sum_xe, rec)

            kt_sb = consts.tile([P, 2, N], BF16)  # K.T [j, m] (pre-scaled)
            for jc in range(2):
                pk = ps_a.tile([P, N], F32, tag="psa", name="pk")
                for kc in range(2):
                    nc.tensor.matmul(
                        pk,
                        wkt_sb[:, kc, jc * P : (jc + 1) * P],
                        tatt_t[:, kc, :],
                        start=(kc == 0),
                        stop=(kc == 1),
                    )
                nc.vector.tensor_copy(kt_sb[:, jc, :], pk)

            v_sb = consts.tile([P, 2, D], BF16)  # V [m, j] (A@V stationary)
            for mc in range(2):
                pv = ps_a.tile([P, D], F32, tag="psa", name="pv")
                for kc in range(2):
                    nc.tensor.matmul(
                        pv,
                        tatt_t[:, kc, mc * P : (mc + 1) * P],
                        wvt_sb[:, kc, :],
                        start=(kc == 0),
                        stop=(kc == 1),
                    )
                nc.vector.tensor_copy(v_sb[:, mc, :], pv)
                # Ship V to the host for the +V self-loop term (DMA cannot
                # read PSUM, so stage through SBUF).
                v_stage = misc.tile([P, D], F32, name=f"vs{mc}")
                nc.scalar.activation(v_stage, pv, AF.Copy)
                nc.sync.dma_start(out=v_d[mc], in_=v_stage)

            # ---------------- Phase C: attention + output
            relu_rr = 0
            for c in range(NCHUNKS):
                cs = slice(c * CHUNK_TN, (c + 1) * CHUNK_TN)
                a_str = {}
                for hg in range(2):
                    for mc in range(2):
                        for rp in range(2):  # head pairs share a 2-bank tile
                            ps2 = ps_a.tile(
                                [P, 2 * CHUNK_TN],
                                F32,
                                tag="psa",
                                name=f"ps{hg}{mc}{rp}",
                            )
                            for rh in range(2):
                                r = rp * 2 + rh
                                nc.tensor.matmul(
                                    ps2[:, rh * CHUNK_TN : (rh + 1) * CHUNK_TN],
                                    kt_sb[
                                        r * 32 : (r + 1) * 32,
                                        hg,
                                        mc * P : (mc + 1) * P,
                                    ],
                                    qt_sb[r * 32 : (r + 1) * 32, hg, cs],
                                    start=True,
                                    stop=True,
                                    tile_position=(r * 32, 0),
                                )
                            a2 = a_pool.tile(
                                [P, 2 * CHUNK_TN],
                                BF16,
                                tag="at",
                                name=f"a{hg}{mc}{rp}",
                            )
                            sel = _RELU_SCHED[relu_rr % len(_RELU_SCHED)]
                            relu_rr += 1
                            if sel == 0:
                                nc.scalar.activation(a2, ps2, AF.Relu)
                            else:
                                nc.vector.tensor_scalar_max(a2, ps2, 0.0)
                            for rh in range(2):
                                a_str[(hg, rp * 2 + rh, mc)] = a2[
                                    :, rh * CHUNK_TN : (rh + 1) * CHUNK_TN
                                ]
                for hg in range(2):
                    po = ps_o.tile([P, CHUNK_TN], F32, tag="po", name=f"po{hg}")
                    # All four column tiles accumulate concurrently into
                    # disjoint partition quadrants of one PSUM bank.
                    for mc in range(2):
                        for r in range(4):
                            h = hg * 4 + r
                            nc.tensor.matmul(
                                po[r * 32 : (r + 1) * 32, :],
                                v_sb[:, mc, h * 32 : (h + 1) * 32],
                                a_str[(hg, r, mc)],
                                start=(mc == 0),
                                stop=(mc == 1),
                                tile_position=(0, r * 32),
                                skip_group_check=True,
                            )
                    # Plain PSUM->SBUF copy (the host adds the +V term),
                    # then contiguous DMA to HBM.
                    o_sb = o_pool.tile([P, CHUNK_TN], F32, tag="ot", name=f"o{hg}")
                    if hg == 0:
                        nc.scalar.activation(o_sb, po, AF.Copy)
                    else:
                        nc.vector.tensor_copy(o_sb, po)
                    dma_eng = nc.gpsimd if hg == 0 else nc.sync
                    dma_eng.dma_start(out=out_v[:, hg, cs], in_=o_sb)

    nc.finalize()
    return nc


def prepare_in_maps(inputs):
    x = np.asarray(inputs["x"], dtype=np.float32)
    w_q = np.asarray(inputs["W_Q"], dtype=np.float32)
    w_k = np.asarray(inputs["W_K"], dtype=np.float32)
    w_v = np.asarray(inputs["W_V"], dtype=np.float32)

    import ml_dtypes

    wqt = np.ascontiguousarray(w_q.T).astype(ml_dtypes.bfloat16)
    wkt = np.ascontiguousarray(w_k.T) * np.float32(1.0 / np.sqrt(DH))
    wvt = np.ascontiguousarray(w_v.T)

    return [
        {
            "xt": np.ascontiguousarray(x[b].reshape(T * N, D).T).astype(
                ml_dtypes.bfloat16
            ),
            "wqt": wqt,
            "wkt": wkt,
            "wvt": wvt,
        }
        for b in range(B)
    ]


def finish_out(res):
    # out.T [D, T*N] -> [T, N, D] per core (+ V self-loop), stacked over B.
    outs = []
    for b in range(B):
        o = res.results[b]["out"].reshape(D, T, N).transpose(1, 2, 0)
        v = res.results[b]["v"].reshape(N, D)
        outs.append(o + v[None, :, :])
    return np.stack(outs, axis=0)


def kernel(**inputs) -> np.ndarray:
    if "nc" not in _CACHE:
        _CACHE["nc"] = _build_program()
    nc = _CACHE["nc"]
    in_maps = prepare_in_maps(inputs)
    res = run_bass_kernel_spmd(nc, in_maps, core_ids=list(range(B)))
    return finish_out(res)


# revision 20
# speedup vs baseline: 1.1375x; 1.0955x over previous
"""Trainium2 Bass kernel for nn_MultiHeadCDGCN.

Math (per batch b):
  t_w  = softmax(x, axis=T);  TAtt = sum_T(x * t_w)          [N, D]
  Q    = x @ W_Q.T                                           [T, N, D]
  K    = TAtt @ W_K.T ; V = TAtt @ W_V.T                     [N, D]
  S_th = Q_th @ K_h.T / sqrt(dh)   (per t, head h)           [N, N]
  out  = (relu(S) + I) @ V = relu(S) @ V + V                 [T, N, D]

Sharding: data-parallel over B across 8 NeuronCores (B == 8, one batch
per core); no collectives.

Key layout choices (v3):
  - Host supplies x.T ([D, T*N], bf16) per core and receives out.T
    ([D, T*N], fp32) plus V; the final transpose and the +V self-loop
    term run on the CPU. On-device this removes all transposes of x,
    the output stream-transpose, and the whole +V evacuation pass: the
    A@V PSUM tiles are DMA'd straight to HBM.
  - Q.T is evacuated from PSUM to SBUF by DMA (no ACT/DVE copy).
  - fp32 matmuls run as float32r; the Q projection and A@V run bf16.
  - S lands in bf16 PSUM tiles of 4 heads ([128, 2048], one bank pair)
    so relu evacuation runs half the instructions and the DVE tiles hit
    the 2-byte 2x path.
  - Softmax-pool stats are bf16 (exp on ACT, x*e mul on DVE 2x, e-sum
    on Pool, xe-sum on DVE) accumulating into fp32.
"""

import sys

import numpy as np

sys.path.insert(0, "/opt/trn_rl_repo")

import concourse.bacc as bacc  # noqa: E402
import concourse.tile as tile  # noqa: E402
from concourse import mybir  # noqa: E402
from concourse.bass_utils import run_bass_kernel_spmd  # noqa: E402

F32 = mybir.dt.float32
F32R = mybir.dt.float32r
BF16 = mybir.dt.bfloat16
AF = mybir.ActivationFunctionType
ALU = mybir.AluOpType

B, T, N, D, H, DH = 8, 32, 256, 256, 8, 32
P = 128
NCHUNKS = 16  # tn chunks of 512 (2 frames each)
CHUNK_T = 2  # frames per chunk
CHUNK_TN = CHUNK_T * N  # 512

_CACHE: dict = {}

# relu evacuation round-robin: 0=ACT, 1=DVE (Pool cannot access PSUM).
_RELU_SCHED = [0, 1, 0, 1, 0, 1, 0, 1]


def _build_program():
    nc = bacc.Bacc()

    xt_d = nc.dram_tensor("xt", [D, T * N], BF16, kind="ExternalInput")
    wqt_d = nc.dram_tensor("wqt", [D, D], BF16, kind="ExternalInput")
    wkt_d = nc.dram_tensor("wkt", [D, D], F32, kind="ExternalInput")
    wvt_d = nc.dram_tensor("wvt", [D, D], F32, kind="ExternalInput")
    out_d = nc.dram_tensor("out", [D, T * N], F32, kind="ExternalOutput")
    v_d = nc.dram_tensor("v", [2, P, D], F32, kind="ExternalOutput")

    xt_v = xt_d.rearrange("(dc p) tn -> p dc tn", p=P)
    out_v = out_d.rearrange("(hg p) tn -> p hg tn", p=P)

    with tile.TileContext(nc) as tc:
        with (
            tc.tile_pool(name="consts", bufs=1) as consts,
            tc.tile_pool(name="ew", bufs=3) as e_pool,
            tc.tile_pool(name="at", bufs=12) as a_pool,
            tc.tile_pool(name="ot", bufs=4) as o_pool,
            tc.tile_pool(name="misc", bufs=1) as misc,
            tc.tile_pool(name="ps_a", bufs=3, space="PSUM") as ps_a,
            tc.tile_pool(name="ps_o", bufs=2, space="PSUM") as ps_o,
        ):
            # Weights, [k, j] with k split over 2 partition tiles.
            wqt_sb = consts.tile([P, 2, D], BF16)
            wkt_sb = consts.tile([P, 2, D], F32R)
            wvt_sb = consts.tile([P, 2, D], F32R)
            for w_sb, w_d in ((wqt_sb, wqt_d), (wkt_sb, wkt_d), (wvt_sb, wvt_d)):
                nc.sync.dma_start(
                    out=w_sb,
                    in_=w_d.rearrange("(kc p) j -> p kc j", p=P).bitcast(
                        w_sb.dtype
                    ),
                )

            # x.T resident: [d%128, d//128, tn] (bf16).
            xt_all = consts.tile([P, 2, T * N], BF16)

            # Q.T strip [j, tn] resident (j split over 2 partition tiles),
            # cast to bf16 during PSUM evacuation so S runs as bf16.
            qt_sb = consts.tile([P, 2, T * N], BF16)

            # Softmax-pool statistics, frames of a chunk kept separate:
            # [d%128, d//128, (ti n)]. All-bf16 accumulation: a DVE
            # tensor_add with any fp32 operand runs ~2.8us per [f1024]
            # vs ~0.7-1.2us all-bf16. Two interleaved accumulators per
            # stat (8 adds each) bound the bf16 rounding accumulation;
            # the fold to fp32 happens once in phase B.
            se_pp = [
                consts.tile([P, 2, CHUNK_TN], BF16, name=f"se{i}")
                for i in range(2)
            ]
            sxe_pp = [
                consts.tile([P, 2, CHUNK_TN], BF16, name=f"sxe{i}")
                for i in range(2)
            ]
            nc.gpsimd.memset(se_pp[0], 0.0)
            nc.gpsimd.memset(sxe_pp[0], 0.0)

            # ---------------- Phase A: stream x.T, stats, Q.T
            for c in range(NCHUNKS):
                cs = slice(c * CHUNK_TN, (c + 1) * CHUNK_TN)
                xt = xt_all[:, :, cs]
                nc.sync.dma_start(out=xt, in_=xt_v[:, :, cs])

                e_t = e_pool.tile([P, 2, CHUNK_TN], BF16, name="e_t")
                nc.scalar.activation(e_t, xt, AF.Exp)
                xe_t = e_pool.tile([P, 2, CHUNK_TN], BF16, name="xe_t")
                nc.vector.tensor_mul(xe_t, xt, e_t)
                # Both sums on DVE: all-bf16 non-aliased adds run ~0.7us
                # ([f1024] 2x path) vs ~2.5us in-place or mixed-dtype.
                # Ping-pong so the output never aliases an input.
                src_i, dst_i = c % 2, 1 - c % 2
                nc.vector.tensor_add(se_pp[dst_i], se_pp[src_i], e_t)
                nc.vector.tensor_add(sxe_pp[dst_i], sxe_pp[src_i], xe_t)

                # Q.T chunk: [j, tn] = sum_k W_Q.T[k, j]^T x.T[k, tn]
                for jc in range(2):
                    pq = ps_a.tile([P, CHUNK_TN], F32, tag="psa", name=f"pq{jc}")
                    for kc in range(2):
                        nc.tensor.matmul(
                            pq,
                            wqt_sb[:, kc, jc * P : (jc + 1) * P],
                            xt[:, kc, :],
                            start=(kc == 0),
                            stop=(kc == 1),
                        )
                    # Both Q.T evacuations on ACT: DVE is the phase-A
                    # bottleneck (2-input stats ops), ACT has slack.
                    nc.scalar.activation(qt_sb[:, jc, cs], pq, AF.Copy)

            # ---------------- Phase B: TAtt.T, K.T, V
            # Fold the two frame-lanes of the stats, then TAtt = sxe/se.
            # The last ping-pong write holds the full sums (bf16).
            sum2_e = se_pp[1 - (NCHUNKS - 1) % 2]
            sum2_xe = sxe_pp[1 - (NCHUNKS - 1) % 2]
            sum_e = misc.tile([P, 2, N], F32)
            sum_xe = misc.tile([P, 2, N], F32)
            nc.vector.tensor_add(sum_e, sum2_e[:, :, :N], sum2_e[:, :, N:])
            nc.scalar.activation(sum_xe, sum2_xe[:, :, :N], AF.Copy)
            nc.vector.tensor_add(sum_xe, sum_xe, sum2_xe[:, :, N:])
            rec = misc.tile([P, 2, N], F32)
            tatt_t = consts.tile([P, 2, N], F32R)  # TAtt.T [d, n]
            nc.vector.reciprocal(rec, sum_e)
            nc.vector.tensor_mul(tatt_t, sum_xe, rec)

            kt_sb = consts.tile([P, 2, N], BF16)  # K.T [j, m] (pre-scaled)
            for jc in range(2):
                pk = ps_a.tile([P, N], F32, tag="psa", name="pk")
                for kc in range(2):
                    nc.tensor.matmul(
                        pk,
                        wkt_sb[:, kc, jc * P : (jc + 1) * P],
                        tatt_t[:, kc, :],
                        start=(kc == 0),
                        stop=(kc == 1),
                    )
                nc.vector.tensor_copy(kt_sb[:, jc, :], pk)

            v_sb = consts.tile([P, 2, D], BF16)  # V [m, j] (A@V stationary)
            for mc in range(2):
                pv = ps_a.tile([P, D], F32, tag="psa", name="pv")
                for kc in range(2):
                    nc.tensor.matmul(
                        pv,
                        tatt_t[:, kc, mc * P : (mc + 1) * P],
                        wvt_sb[:, kc, :],
                        start=(kc == 0),
                        stop=(kc == 1),
                    )
                nc.vector.tensor_copy(v_sb[:, mc, :], pv)
                # Ship V to the host for the +V self-loop term (DMA cannot
                # read PSUM, so stage through SBUF).
                v_stage = misc.tile([P, D], F32, name=f"vs{mc}")
                nc.scalar.activation(v_stage, pv, AF.Copy)
                nc.sync.dma_start(out=v_d[mc], in_=v_stage)

            # ---------------- Phase C: attention + output
            relu_rr = 0
            for c in range(NCHUNKS):
                cs = slice(c * CHUNK_TN, (c + 1) * CHUNK_TN)
                a_str = {}
                for hg in range(2):
                    for mc in range(2):
                        for rp in range(2):  # head pairs share a 2-bank tile
                            ps2 = ps_a.tile(
                                [P, 2 * CHUNK_TN],
                                F32,
                                tag="psa",
                                name=f"ps{hg}{mc}{rp}",
                            )
                            for rh in range(2):
                                r = rp * 2 + rh
                                nc.tensor.matmul(
                                    ps2[:, rh * CHUNK_TN : (rh + 1) * CHUNK_TN],
                                    kt_sb[
                                        r * 32 : (r + 1) * 32,
                                        hg,
                                        mc * P : (mc + 1) * P,
                                    ],
                                    qt_sb[r * 32 : (r + 1) * 32, hg, cs],
                                    start=True,
                                    stop=True,
                                    tile_position=(r * 32, 0),
                                )
                            a2 = a_pool.tile(
                                [P, 2 * CHUNK_TN],
                                BF16,
                                tag="at",
                                name=f"a{hg}{mc}{rp}",
                            )
                            sel = _RELU_SCHED[relu_rr % len(_RELU_SCHED)]
                            relu_rr += 1
                            if sel == 0:
                                nc.scalar.activation(a2, ps2, AF.Relu)
                            else:
                                nc.vector.tensor_scalar_max(a2, ps2, 0.0)
                            for rh in range(2):
                                a_str[(hg, rp * 2 + rh, mc)] = a2[
                                    :, rh * CHUNK_TN : (rh + 1) * CHUNK_TN
                                ]
                for hg in range(2):
                    po = ps_o.tile([P, CHUNK_TN], F32, tag="po", name=f"po{hg}")
                    # All four column tiles accumulate concurrently into
                    # disjoint partition quadrants of one PSUM bank.
                    for mc in range(2):
                        for r in range(4):
                            h = hg * 4 + r
                            nc.tensor.matmul(
                                po[r * 32 : (r + 1) * 32, :],
                                v_sb[:, mc, h * 32 : (h + 1) * 32],
                                a_str[(hg, r, mc)],
                                start=(mc == 0),
                                stop=(mc == 1),
                                tile_position=(0, r * 32),
                                skip_group_check=True,
                            )
                    # Plain PSUM->SBUF copy (the host adds the +V term),
                    # then contiguous DMA to HBM.
                    o_sb = o_pool.tile([P, CHUNK_TN], F32, tag="ot", name=f"o{hg}")
                    if hg == 0:
                        nc.scalar.activation(o_sb, po, AF.Copy)
                    else:
                        nc.vector.tensor_copy(o_sb, po)
                    dma_eng = nc.gpsimd if hg == 0 else nc.sync
                    dma_eng.dma_start(out=out_v[:, hg, cs], in_=o_sb)

    nc.finalize()
    return nc


def prepare_in_maps(inputs):
    x = np.asarray(inputs["x"], dtype=np.float32)
    w_q = np.asarray(inputs["W_Q"], dtype=np.float32)
    w_k = np.asarray(inputs["W_K"], dtype=np.float32)
    w_v = np.asarray(inputs["W_V"], dtype=np.float32)

    import ml_dtypes

    wqt = np.ascontiguousarray(w_q.T).astype(ml_dtypes.bfloat16)
    wkt = np.ascontiguousarray(w_k.T) * np.float32(1.0 / np.sqrt(DH))
    wvt = np.ascontiguousarray(w_v.T)

    return [
        {
            "xt": np.ascontiguousarray(x[b].reshape(T * N, D).T).astype(
                ml_dtypes.bfloat16
            ),
            "wqt": wqt,
            "wkt": wkt,
            "wvt": wvt,
        }
        for b in range(B)
    ]


def finish_out(res):
    # out.T [D, T*N] -> [T, N, D] per core (+ V self-loop), stacked over B.
    outs = []
    for b in range(B):
        o = res.results[b]["out"].reshape(D, T, N).transpose(1, 2, 0)
        v = res.results[b]["v"].reshape(N, D)
        outs.append(o + v[None, :, :])
    return np.stack(outs, axis=0)


def kernel(**inputs) -> np.ndarray:
    if "nc" not in _CACHE:
        _CACHE["nc"] = _build_program()
    nc = _CACHE["nc"]
    in_maps = prepare_in_maps(inputs)
    res = run_bass_kernel_spmd(nc, in_maps, core_ids=list(range(B)))
    return finish_out(res)


# revision 21
# speedup vs baseline: 1.1694x; 1.0280x over previous
"""Trainium2 Bass kernel for nn_MultiHeadCDGCN.

Math (per batch b):
  t_w  = softmax(x, axis=T);  TAtt = sum_T(x * t_w)          [N, D]
  Q    = x @ W_Q.T                                           [T, N, D]
  K    = TAtt @ W_K.T ; V = TAtt @ W_V.T                     [N, D]
  S_th = Q_th @ K_h.T / sqrt(dh)   (per t, head h)           [N, N]
  out  = (relu(S) + I) @ V = relu(S) @ V + V                 [T, N, D]

Sharding: data-parallel over B across 8 NeuronCores (B == 8, one batch
per core); no collectives.

Key layout choices (v3):
  - Host supplies x.T ([D, T*N], bf16) per core and receives out.T
    ([D, T*N], fp32) plus V; the final transpose and the +V self-loop
    term run on the CPU. On-device this removes all transposes of x,
    the output stream-transpose, and the whole +V evacuation pass: the
    A@V PSUM tiles are DMA'd straight to HBM.
  - Q.T is evacuated from PSUM to SBUF by DMA (no ACT/DVE copy).
  - fp32 matmuls run as float32r; the Q projection and A@V run bf16.
  - S lands in bf16 PSUM tiles of 4 heads ([128, 2048], one bank pair)
    so relu evacuation runs half the instructions and the DVE tiles hit
    the 2-byte 2x path.
  - Softmax-pool stats are bf16 (exp on ACT, x*e mul on DVE 2x, e-sum
    on Pool, xe-sum on DVE) accumulating into fp32.
"""

import sys

import numpy as np

sys.path.insert(0, "/opt/trn_rl_repo")

import concourse.bacc as bacc  # noqa: E402
import concourse.tile as tile  # noqa: E402
from concourse import mybir  # noqa: E402
from concourse.bass_utils import run_bass_kernel_spmd  # noqa: E402

F32 = mybir.dt.float32
F32R = mybir.dt.float32r
BF16 = mybir.dt.bfloat16
AF = mybir.ActivationFunctionType
ALU = mybir.AluOpType

B, T, N, D, H, DH = 8, 32, 256, 256, 8, 32
P = 128
NCHUNKS = 16  # tn chunks of 512 (2 frames each)
CHUNK_T = 2  # frames per chunk
CHUNK_TN = CHUNK_T * N  # 512

_CACHE: dict = {}

# relu evacuation round-robin: 0=ACT, 1=DVE (Pool cannot access PSUM).
_RELU_SCHED = [0, 1, 0, 1, 0, 1, 0, 1]


def _build_program():
    nc = bacc.Bacc()

    xt_d = nc.dram_tensor("xt", [D, T * N], BF16, kind="ExternalInput")
    wqt_d = nc.dram_tensor("wqt", [D, D], BF16, kind="ExternalInput")
    wkt_d = nc.dram_tensor("wkt", [D, D], F32, kind="ExternalInput")
    wvt_d = nc.dram_tensor("wvt", [D, D], F32, kind="ExternalInput")
    out_d = nc.dram_tensor("out", [D, T * N], F32, kind="ExternalOutput")
    v_d = nc.dram_tensor("v", [2, P, D], F32, kind="ExternalOutput")

    xt_v = xt_d.rearrange("(dc p) tn -> p dc tn", p=P)
    out_v = out_d.rearrange("(hg p) tn -> p hg tn", p=P)

    with tile.TileContext(nc) as tc:
        with (
            tc.tile_pool(name="consts", bufs=1) as consts,
            tc.tile_pool(name="ew", bufs=3) as e_pool,
            tc.tile_pool(name="at", bufs=12) as a_pool,
            tc.tile_pool(name="ot", bufs=4) as o_pool,
            tc.tile_pool(name="misc", bufs=1) as misc,
            tc.tile_pool(name="ps_a", bufs=3, space="PSUM") as ps_a,
            tc.tile_pool(name="ps_o", bufs=2, space="PSUM") as ps_o,
        ):
            # Weights, [k, j] with k split over 2 partition tiles.
            wqt_sb = consts.tile([P, 2, D], BF16)
            wkt_sb = consts.tile([P, 2, D], F32R)
            wvt_sb = consts.tile([P, 2, D], F32R)
            for w_sb, w_d in ((wqt_sb, wqt_d), (wkt_sb, wkt_d), (wvt_sb, wvt_d)):
                nc.sync.dma_start(
                    out=w_sb,
                    in_=w_d.rearrange("(kc p) j -> p kc j", p=P).bitcast(
                        w_sb.dtype
                    ),
                )

            # x.T resident: [d%128, d//128, tn] (bf16).
            xt_all = consts.tile([P, 2, T * N], BF16)

            # Q.T strip [j, tn] resident (j split over 2 partition tiles),
            # cast to bf16 during PSUM evacuation so S runs as bf16.
            qt_sb = consts.tile([P, 2, T * N], BF16)

            # Softmax-pool statistics, frames of a chunk kept separate:
            # [d%128, d//128, (ti n)]. All-bf16 accumulation: a DVE
            # tensor_add with any fp32 operand runs ~2.8us per [f1024]
            # vs ~0.7-1.2us all-bf16. Two interleaved accumulators per
            # stat (8 adds each) bound the bf16 rounding accumulation;
            # the fold to fp32 happens once in phase B.
            se_pp = [
                consts.tile([P, 2, CHUNK_TN], BF16, name=f"se{i}")
                for i in range(2)
            ]
            sxe_pp = [
                consts.tile([P, 2, CHUNK_TN], BF16, name=f"sxe{i}")
                for i in range(2)
            ]
            nc.gpsimd.memset(se_pp[0], 0.0)
            nc.gpsimd.memset(sxe_pp[0], 0.0)

            # ---------------- Phase A: stream x.T, stats, Q.T
            for c in range(NCHUNKS):
                cs = slice(c * CHUNK_TN, (c + 1) * CHUNK_TN)
                xt = xt_all[:, :, cs]
                nc.sync.dma_start(out=xt, in_=xt_v[:, :, cs])

                e_t = e_pool.tile([P, 2, CHUNK_TN], BF16, name="e_t")
                nc.scalar.activation(e_t, xt, AF.Exp)
                xe_t = e_pool.tile([P, 2, CHUNK_TN], BF16, name="xe_t")
                nc.vector.tensor_mul(xe_t, xt, e_t)
                # Both sums on DVE: all-bf16 non-aliased adds run ~0.7us
                # ([f1024] 2x path) vs ~2.5us in-place or mixed-dtype.
                # Ping-pong so the output never aliases an input.
                src_i, dst_i = c % 2, 1 - c % 2
                nc.vector.tensor_add(se_pp[dst_i], se_pp[src_i], e_t)
                nc.vector.tensor_add(sxe_pp[dst_i], sxe_pp[src_i], xe_t)

                # Q.T chunk: [j, tn] = sum_k W_Q.T[k, j]^T x.T[k, tn].
                # Both jc halves land in one 2-bank PSUM tile so a single
                # ACT copy evacuates the whole chunk.
                pq = ps_a.tile([P, 2, CHUNK_TN], F32, tag="psa", name="pq")
                for jc in range(2):
                    for kc in range(2):
                        nc.tensor.matmul(
                            pq[:, jc, :],
                            wqt_sb[:, kc, jc * P : (jc + 1) * P],
                            xt[:, kc, :],
                            start=(kc == 0),
                            stop=(kc == 1),
                        )
                nc.scalar.activation(qt_sb[:, :, cs], pq, AF.Copy)

            # ---------------- Phase B: TAtt.T, K.T, V
            # Fold the two frame-lanes of the stats, then TAtt = sxe/se.
            # The last ping-pong write holds the full sums (bf16).
            sum2_e = se_pp[1 - (NCHUNKS - 1) % 2]
            sum2_xe = sxe_pp[1 - (NCHUNKS - 1) % 2]
            sum_e = misc.tile([P, 2, N], F32)
            sum_xe = misc.tile([P, 2, N], F32)
            nc.vector.tensor_add(sum_e, sum2_e[:, :, :N], sum2_e[:, :, N:])
            nc.scalar.activation(sum_xe, sum2_xe[:, :, :N], AF.Copy)
            nc.vector.tensor_add(sum_xe, sum_xe, sum2_xe[:, :, N:])
            rec = misc.tile([P, 2, N], F32)
            tatt_t = consts.tile([P, 2, N], F32R)  # TAtt.T [d, n]
            nc.vector.reciprocal(rec, sum_e)
            nc.vector.tensor_mul(tatt_t, sum_xe, rec)

            kt_sb = consts.tile([P, 2, N], BF16)  # K.T [j, m] (pre-scaled)
            for jc in range(2):
                pk = ps_a.tile([P, N], F32, tag="psa", name="pk")
                for kc in range(2):
                    nc.tensor.matmul(
                        pk,
                        wkt_sb[:, kc, jc * P : (jc + 1) * P],
                        tatt_t[:, kc, :],
                        start=(kc == 0),
                        stop=(kc == 1),
                    )
                nc.vector.tensor_copy(kt_sb[:, jc, :], pk)

            v_sb = consts.tile([P, 2, D], BF16)  # V [m, j] (A@V stationary)
            for mc in range(2):
                pv = ps_a.tile([P, D], F32, tag="psa", name="pv")
                for kc in range(2):
                    nc.tensor.matmul(
                        pv,
                        tatt_t[:, kc, mc * P : (mc + 1) * P],
                        wvt_sb[:, kc, :],
                        start=(kc == 0),
                        stop=(kc == 1),
                    )
                nc.vector.tensor_copy(v_sb[:, mc, :], pv)
                # Ship V to the host for the +V self-loop term (DMA cannot
                # read PSUM, so stage through SBUF).
                v_stage = misc.tile([P, D], F32, name=f"vs{mc}")
                nc.scalar.activation(v_stage, pv, AF.Copy)
                nc.sync.dma_start(out=v_d[mc], in_=v_stage)

            # ---------------- Phase C: attention + output
            relu_rr = 0
            for c in range(NCHUNKS):
                cs = slice(c * CHUNK_TN, (c + 1) * CHUNK_TN)
                a_str = {}
                for hg in range(2):
                    for mc in range(2):
                        for rp in range(2):  # head pairs share a 2-bank tile
                            ps2 = ps_a.tile(
                                [P, 2 * CHUNK_TN],
                                F32,
                                tag="psa",
                                name=f"ps{hg}{mc}{rp}",
                            )
                            for rh in range(2):
                                r = rp * 2 + rh
                                nc.tensor.matmul(
                                    ps2[:, rh * CHUNK_TN : (rh + 1) * CHUNK_TN],
                                    kt_sb[
                                        r * 32 : (r + 1) * 32,
                                        hg,
                                        mc * P : (mc + 1) * P,
                                    ],
                                    qt_sb[r * 32 : (r + 1) * 32, hg, cs],
                                    start=True,
                                    stop=True,
                                    tile_position=(r * 32, 0),
                                )
                            a2 = a_pool.tile(
                                [P, 2 * CHUNK_TN],
                                BF16,
                                tag="at",
                                name=f"a{hg}{mc}{rp}",
                            )
                            sel = _RELU_SCHED[relu_rr % len(_RELU_SCHED)]
                            relu_rr += 1
                            if sel == 0:
                                nc.scalar.activation(a2, ps2, AF.Relu)
                            else:
                                nc.vector.tensor_scalar_max(a2, ps2, 0.0)
                            for rh in range(2):
                                a_str[(hg, rp * 2 + rh, mc)] = a2[
                                    :, rh * CHUNK_TN : (rh + 1) * CHUNK_TN
                                ]
                for hg in range(2):
                    po = ps_o.tile([P, CHUNK_TN], F32, tag="po", name=f"po{hg}")
                    # All four column tiles accumulate concurrently into
                    # disjoint partition quadrants of one PSUM bank.
                    for mc in range(2):
                        for r in range(4):
                            h = hg * 4 + r
                            nc.tensor.matmul(
                                po[r * 32 : (r + 1) * 32, :],
                                v_sb[:, mc, h * 32 : (h + 1) * 32],
                                a_str[(hg, r, mc)],
                                start=(mc == 0),
                                stop=(mc == 1),
                                tile_position=(0, r * 32),
                                skip_group_check=True,
                            )
                    # Plain PSUM->SBUF copy (the host adds the +V term),
                    # then contiguous DMA to HBM.
                    o_sb = o_pool.tile([P, CHUNK_TN], F32, tag="ot", name=f"o{hg}")
                    if hg == 0:
                        nc.scalar.activation(o_sb, po, AF.Copy)
                    else:
                        nc.vector.tensor_copy(o_sb, po)
                    dma_eng = nc.gpsimd if hg == 0 else nc.sync
                    dma_eng.dma_start(out=out_v[:, hg, cs], in_=o_sb)

    nc.finalize()
    return nc


def prepare_in_maps(inputs):
    x = np.asarray(inputs["x"], dtype=np.float32)
    w_q = np.asarray(inputs["W_Q"], dtype=np.float32)
    w_k = np.asarray(inputs["W_K"], dtype=np.float32)
    w_v = np.asarray(inputs["W_V"], dtype=np.float32)

    import ml_dtypes

    wqt = np.ascontiguousarray(w_q.T).astype(ml_dtypes.bfloat16)
    wkt = np.ascontiguousarray(w_k.T) * np.float32(1.0 / np.sqrt(DH))
    wvt = np.ascontiguousarray(w_v.T)

    return [
        {
            "xt": np.ascontiguousarray(x[b].reshape(T * N, D).T).astype(
                ml_dtypes.bfloat16
            ),
            "wqt": wqt,
            "wkt": wkt,
            "wvt": wvt,
        }
        for b in range(B)
    ]


def finish_out(res):
    # out.T [D, T*N] -> [T, N, D] per core (+ V self-loop), stacked over B.
    outs = []
    for b in range(B):
        o = res.results[b]["out"].reshape(D, T, N).transpose(1, 2, 0)
        v = res.results[b]["v"].reshape(N, D)
        outs.append(o + v[None, :, :])
    return np.stack(outs, axis=0)


def kernel(**inputs) -> np.ndarray:
    if "nc" not in _CACHE:
        _CACHE["nc"] = _build_program()
    nc = _CACHE["nc"]
    in_maps = prepare_in_maps(inputs)
    res = run_bass_kernel_spmd(nc, in_maps, core_ids=list(range(B)))
    return finish_out(res)
